# revision 15
# baseline (speedup 1.0000x reference)
"""DualAttentionEdgeGAT Trainium2 kernel.

Strategy:
  - Sort edges by target node (idx_i); split nodes into 8 contiguous ranges with
    ~equal edge counts -> each core owns a node range and ALL edges targeting it.
    No collectives needed: the segment-max over idx_i is core-local.
  - Per core, bin-pack its nodes into 128 partitions (whole nodes per partition)
    with capacity C slots; pad partitions with duplicate edges. Device edge order
    e = t*128 + p  (slot t of partition p).
  - Feature-major compute layout on device: (feature-on-partition, edge-on-free).
    x rows are gathered on-device with dma_gather(transpose=True) on a bf16 copy
    of x. edge_feature / geo_features are host-pre-gathered+transposed per shard.
  - Per-edge scalar s is computed per group of 512 edges; segment-max is done
    with per-node additive-mask max-reductions (offs masks from host); the prop
    MLP runs node-sharded with the per-node max entering as a rank-1 (K=1)
    matmul term.
"""

import numpy as np
import ml_dtypes

import concourse.bass as bass
import concourse.tile as tile
from concourse import bacc, mybir
from concourse import bass_utils

F32 = mybir.dt.float32
BF16 = mybir.dt.bfloat16
I16 = mybir.dt.int16
AF = mybir.ActivationFunctionType
ALU = mybir.AluOpType

N, E = 20000, 100000
DN = DE = DA = 256
H = 8
dn = de = do = 32
T = 512          # edges per group
P = 128
NEG = -1.0e30

_cache = {}
_last_results = None
_last_metas = None


# --------------------------------------------------------------------------
# host-side preparation
# --------------------------------------------------------------------------

def _plan(idx_i):
    """Global plan: per-core node ranges, per-core bin packing, edge order."""
    order = np.argsort(idx_i, kind="stable")
    deg = np.bincount(idx_i, minlength=N)
    starts = np.zeros(N + 1, np.int64)
    np.cumsum(deg, out=starts[1:])

    # split nodes into 8 ranges with ~equal edges
    cum = starts[1:]
    bounds = [0]
    for c in range(1, 8):
        target = E * c / 8.0
        bounds.append(int(np.searchsorted(cum, target)))
    bounds.append(N)

    cores = []
    Cmax, Jmax = 0, 0
    for c in range(8):
        n0, n1 = bounds[c], bounds[c + 1]
        nodes = np.arange(n0, n1)
        nd = deg[n0:n1]
        # snake assignment of degree-sorted nodes to 128 bins
        o = np.argsort(-nd, kind="stable")
        bins_nodes = [[] for _ in range(P)]
        bins_load = np.zeros(P, np.int64)
        for i, k in enumerate(o):
            r = i % (2 * P)
            b = r if r < P else 2 * P - 1 - r
            bins_nodes[b].append(nodes[k])
            bins_load[b] += nd[k]
        C = int(bins_load.max())
        J = max(len(b) for b in bins_nodes)
        cores.append((n0, n1, bins_nodes))
        Cmax = max(Cmax, C)
        Jmax = max(Jmax, J)

    C = ((Cmax + 3) // 4 + 0) * 4
    # groups of T edges need 128*C % T == 0  ->  C % 4 == 0 (T=512)
    J = ((Jmax + 3) // 4) * 4   # NP = J*128 divisible by 512
    return order, starts, deg, cores, C, J


def _prep_core(core, order, starts, deg, C, J, idx_i, idx_j,
               edge_feature, geo_features, x32):
    n0, n1, bins_nodes = core
    EC = P * C
    NP = J * P

    slot_edge = np.zeros((P, C), np.int64)       # global edge id per slot
    slot_node = np.full((P, C), -1, np.int64)    # local node j per slot
    node_of_slotj = np.full((P, J), -1, np.int64)  # global node id per (p, j)

    for p in range(P):
        t = 0
        for j, n in enumerate(bins_nodes[p]):
            node_of_slotj[p, j] = n
            d = deg[n]
            if d:
                slot_edge[p, t:t + d] = order[starts[n]:starts[n] + d]
                slot_node[p, t:t + d] = j
                t += d
        if t == 0:
            slot_edge[p, :] = order[0]   # fully padded bin
        else:
            slot_edge[p, t:] = slot_edge[p, t - 1]

    dev_edges = slot_edge.T.reshape(-1)          # e = t*128 + p

    def wrap16(v):
        w = np.zeros((16, EC // 16), np.int16)
        idx = np.arange(EC)
        w[idx % 16, idx // 16] = v.astype(np.int16)
        return np.tile(w, (8, 1))

    xi16 = wrap16(idx_i[dev_edges])
    xj16 = wrap16(idx_j[dev_edges])

    efT = np.ascontiguousarray(
        edge_feature[dev_edges].T.reshape(2, P, EC).transpose(1, 0, 2))
    geoT = np.ascontiguousarray(geo_features[dev_edges].T)  # (11, EC)

    offs = np.where(slot_node[:, None, :] == np.arange(J)[None, :, None],
                    np.float32(0.0), np.float32(NEG)).astype(np.float32)

    # prop node slot n = j*128 + p
    xprop = np.zeros((NP, DN), np.float32)
    prop_nodes = np.full(NP, -1, np.int64)
    for p in range(P):
        for j in range(J):
            n = node_of_slotj[p, j] if j < node_of_slotj.shape[1] else -1
            if n >= 0:
                xprop[j * P + p] = x32[n]
                prop_nodes[j * P + p] = n
    xpropT = np.ascontiguousarray(
        xprop.T.reshape(2, P, NP).transpose(1, 0, 2))

    return dict(xi16=xi16, xj16=xj16, efT=efT, geoT=geoT, offs=offs,
                xpropT=xpropT, dev_edges=dev_edges, prop_nodes=prop_nodes)


def _prep_weights(inp):
    """Shared (all-core) weight arrays in device layouts."""
    perm = np.array([[c * 8 + h for c in range(32)] for h in range(8)]).reshape(-1)
    w = {}

    def resh_lhs(W):  # (K, M) -> (128, K/128, M/128, 128)
        K, M = W.shape
        return np.ascontiguousarray(
            W.reshape(K // P, P, M // P, P).transpose(1, 0, 2, 3))

    We1, We2 = inp["We1"], inp["We2"]
    w["We1A"] = resh_lhs(We1[0:256]).astype(ml_dtypes.bfloat16)
    w["We1B"] = resh_lhs(We1[256:512]).astype(np.float32)
    w["We1C"] = resh_lhs(We1[512:768]).astype(ml_dtypes.bfloat16)
    w["We2"] = resh_lhs(We2).astype(ml_dtypes.bfloat16)
    w["be1"] = inp["be1"].reshape(4, P).T.copy()
    w["be2"] = inp["be2"].reshape(2, P).T.copy()

    w["Wq"] = resh_lhs(inp["Wq"][:, perm]).astype(ml_dtypes.bfloat16)
    w["bq"] = inp["bq"][perm].reshape(2, P).T.copy()
    w["Wep"] = resh_lhs(inp["Wep"][:, perm]).astype(np.float32)
    w["bep"] = inp["bep"][perm].reshape(2, P).T.copy()
    w["Wv"] = resh_lhs(inp["Wv"][:, perm]).astype(ml_dtypes.bfloat16)
    w["bv"] = inp["bv"][perm].reshape(2, P).T.copy()
    Wg = inp["Wg"][:, perm]                      # (11, 256)
    w["Wg"] = np.ascontiguousarray(Wg.reshape(11, 2, P))
    w["bg"] = inp["bg"][perm].reshape(2, P).T.copy()

    def rep4(Wt):  # (32, M) -> (128, M)
        return np.tile(Wt, (4, 1)).astype(ml_dtypes.bfloat16)

    Wg1, Wg2 = inp["Wg1"], inp["Wg2"]            # (96,96), (32,96)
    w["Wg1q"] = rep4(Wg1[:, 0:32].T)             # (128, 96)
    w["Wg1e"] = rep4(Wg1[:, 32:64].T)
    w["Wg1g"] = rep4(Wg1[:, 64:96].T)
    w["bg1"] = inp["bg1"].reshape(96, 1).astype(np.float32)
    w["Wg2T"] = Wg2.T.astype(ml_dtypes.bfloat16)                     # (96, 32)
    w["bg2r"] = np.tile(inp["bg2"], 4).reshape(P, 1)
    Ws1, Ws2 = inp["Ws1"], inp["Ws2"]            # (64,64), (32,64)
    w["Ws1q"] = rep4(Ws1[:, 0:32].T)             # (128, 64)
    w["Ws1e"] = rep4(Ws1[:, 32:64].T)
    w["bs1"] = inp["bs1"].reshape(64, 1).astype(np.float32)
    w["Ws2T"] = Ws2.T.astype(ml_dtypes.bfloat16)                     # (64, 32)
    w["bs2r"] = np.tile(inp["bs2"], 4).reshape(P, 1)

    w["Wb1"] = resh_lhs(inp["Wb1"])              # (128,2,1,128)
    w["bb1"] = inp["bb1"].reshape(P, 1)
    w["g1"] = inp["g1"].reshape(P, 1)
    w["beta1"] = inp["beta1"].reshape(P, 1)
    w["Wb2"] = inp["Wb2"].copy()                 # (128, 64)
    w["bb2"] = inp["bb2"].reshape(64, 1)
    w["g2"] = inp["g2"].reshape(64, 1)
    w["beta2"] = inp["beta2"].reshape(64, 1)
    w["Wb3"] = inp["Wb3"].copy()                 # (64, 1)
    w["bb3t"] = inp["bb3"].reshape(1, 1).astype(np.float32)

    SelA = np.zeros((P, 8), np.float32)
    SelB = np.zeros((P, 8), np.float32)
    for p in range(P):
        SelA[p, p // 32] = 1.0
        SelB[p, 4 + p // 32] = 1.0
    w["SelA"], w["SelB"] = SelA.astype(ml_dtypes.bfloat16), SelB.astype(ml_dtypes.bfloat16)
    w["ones8"] = np.ones((8, 1), np.float32)
    w["ones128d"] = np.full((P, 1), 1.0 / 128, np.float32)
    w["ones64d"] = np.full((64, 1), 1.0 / 64, np.float32)
    w["onesbc"] = np.ones((1, P), np.float32)
    w["ident"] = np.eye(P, dtype=np.float32)
    w["ones1"] = np.ones((1, 1), np.float32)

    Wp1, Wp2 = inp["Wp1"], inp["Wp2"]            # (512,512), (512,256)
    w["Wp1a"] = resh_lhs(Wp1[0:256])             # (128,2,4,128)
    w["wp1sum"] = Wp1[256:512].sum(axis=0).reshape(1, 4, P).copy()
    w["bp1"] = inp["bp1"].reshape(4, P).T.copy()
    w["Wp2"] = resh_lhs(Wp2)                     # (128,4,2,128)
    w["bp2"] = inp["bp2"].reshape(2, P).T.copy()
    return w


# --------------------------------------------------------------------------
# device kernel
# --------------------------------------------------------------------------

def _build(EC, J, C, stage=99):
    NP = J * P
    G = EC // T
    GP = NP // T
    nc = bacc.Bacc()

    # ---- dram tensors
    dx = nc.dram_tensor("x16", (N, DN), BF16, kind="ExternalInput")
    dxi = nc.dram_tensor("xi16", (P, EC // 16), I16, kind="ExternalInput")
    dxj = nc.dram_tensor("xj16", (P, EC // 16), I16, kind="ExternalInput")
    def_ = nc.dram_tensor("efT", (P, 2, EC), F32, kind="ExternalInput")
    dgeo = nc.dram_tensor("geoT", (11, EC), F32, kind="ExternalInput")
    doffs = nc.dram_tensor("offs", (P, J, C), F32, kind="ExternalInput")
    dxp = nc.dram_tensor("xpropT", (P, 2, NP), F32, kind="ExternalInput")

    wnames = {}
    wspecs = [
        ("We1A", (P, 2, 4, P), BF16), ("We1B", (P, 2, 4, P), F32),
        ("We1C", (P, 2, 4, P), BF16), ("We2", (P, 4, 2, P), BF16),
        ("be1", (P, 4), F32), ("be2", (P, 2), F32),
        ("Wq", (P, 2, 2, P), BF16), ("bq", (P, 2), F32),
        ("Wep", (P, 2, 2, P), F32), ("bep", (P, 2), F32),
        ("Wv", (P, 2, 2, P), BF16), ("bv", (P, 2), F32),
        ("Wg", (11, 2, P), F32), ("bg", (P, 2), F32),
        ("Wg1q", (P, 96), BF16), ("Wg1e", (P, 96), BF16), ("Wg1g", (P, 96), BF16),
        ("bg1", (96, 1), F32), ("Wg2T", (96, 32), BF16), ("bg2r", (P, 1), F32),
        ("Ws1q", (P, 64), BF16), ("Ws1e", (P, 64), BF16), ("bs1", (64, 1), F32),
        ("Ws2T", (64, 32), BF16), ("bs2r", (P, 1), F32),
        ("Wb1", (P, 2, 1, P), F32), ("bb1", (P, 1), F32),
        ("g1", (P, 1), F32), ("beta1", (P, 1), F32),
        ("Wb2", (P, 64), F32), ("bb2", (64, 1), F32),
        ("g2", (64, 1), F32), ("beta2", (64, 1), F32),
        ("Wb3", (64, 1), F32), ("bb3t", (1, 1), F32),
        ("SelA", (P, 8), BF16), ("SelB", (P, 8), BF16), ("ones8", (8, 1), F32),
        ("ones128d", (P, 1), F32), ("ones64d", (64, 1), F32),
        ("onesbc", (1, P), F32), ("ident", (P, P), F32), ("ones1", (1, 1), F32),
        ("Wp1a", (P, 2, 4, P), F32), ("wp1sum", (1, 4, P), F32),
        ("bp1", (P, 4), F32), ("Wp2", (P, 4, 2, P), F32), ("bp2", (P, 2), F32),
    ]
    for nm, shp, dt in wspecs:
        wnames[nm] = nc.dram_tensor(nm, shp, dt, kind="ExternalInput")

    dgcn = nc.dram_tensor("gcnT", (P, 2, EC), F32, kind="ExternalOutput")
    dbal = nc.dram_tensor("bal", (EC // T, T), F32, kind="ExternalOutput")
    dxx = nc.dram_tensor("xxT", (P, 2, NP), F32, kind="ExternalOutput")
    dsdbg = nc.dram_tensor("sdbg", (EC // T, T), F32, kind="ExternalOutput")
    dgrow = nc.dram_tensor("grow", (1, NP), F32, kind="ExternalOutput")

    bb3 = None  # placed below

    with tile.TileContext(nc) as tc:
        with tc.tile_pool(name="wpool", bufs=1) as wp, \
             tc.tile_pool(name="io", bufs=2) as io, \
             tc.tile_pool(name="work", bufs=2) as wk, \
             tc.tile_pool(name="work1", bufs=1) as wk1, \
             tc.tile_pool(name="acc", bufs=1) as acc, \
             tc.tile_pool(name="pbig", bufs=4, space="PSUM") as pbig, \
             tc.tile_pool(name="pmid", bufs=4, space="PSUM") as pmid:

            # ---- load weights / indices (resident)
            W = {}
            for nm, shp, dt in wspecs:
                t_ = wp.tile(list(shp), dt, name=f"w_{nm}")
                nc.sync.dma_start(out=t_[:], in_=wnames[nm][:])
                W[nm] = t_
            s_pm = acc.tile([P, C], F32, name="s_pm")
            xi_t = wp.tile([P, EC // 16], I16, name="xi_t")
            nc.sync.dma_start(out=xi_t[:], in_=dxi[:])
            xj_t = wp.tile([P, EC // 16], I16, name="xj_t")
            nc.sync.dma_start(out=xj_t[:], in_=dxj[:])


            for g in range(G):
                es = slice(g * T, (g + 1) * T)

                # ---- input tiles
                xiT = io.tile([P, 2, T], BF16, tag="xiT")
                nc.gpsimd.dma_gather(xiT[:], dx[:], xi_t[:, g * (T // 16):(g + 1) * (T // 16)],
                                     num_idxs=T, num_idxs_reg=T, elem_size=DN,
                                     transpose=True)
                xjT = io.tile([P, 2, T], BF16, tag="xjT")
                nc.gpsimd.dma_gather(xjT[:], dx[:], xj_t[:, g * (T // 16):(g + 1) * (T // 16)],
                                     num_idxs=T, num_idxs_reg=T, elem_size=DN,
                                     transpose=True)
                efT = io.tile([P, 2, T], F32, tag="efT")
                nc.sync.dma_start(out=efT[:], in_=def_[:, :, es])
                geoT = io.tile([11, T], F32, tag="geoT")
                nc.sync.dma_start(out=geoT[:], in_=dgeo[:, es])

                # ---- nn_edge layer 1: (768 -> 512), relu
                h_nn = wk.tile([P, 4, T], BF16, tag="h_nn")
                for mc in range(4):
                    ps = pbig.tile([P, T], F32, tag="pb")
                    first = True
                    for kc in range(2):
                        nc.tensor.matmul(out=ps[:], lhsT=W["We1A"][:, kc, mc, :],
                                         rhs=xiT[:, kc, :], start=first, stop=False)
                        first = False
                    for kc in range(2):
                        nc.tensor.matmul(out=ps[:], lhsT=W["We1B"][:, kc, mc, :],
                                         rhs=efT[:, kc, :], start=False, stop=False)
                    for kc in range(2):
                        nc.tensor.matmul(out=ps[:], lhsT=W["We1C"][:, kc, mc, :],
                                         rhs=xjT[:, kc, :], start=False,
                                         stop=(kc == 1))
                    nc.scalar.activation(out=h_nn[:, mc, :], in_=ps[:], func=AF.Relu,
                                         bias=W["be1"][:, mc:mc + 1])

                # ---- nn_edge layer 2: (512 -> 256) + be2 -> gcn out
                gcn_sb = wk.tile([P, 2, T], F32, tag="gcn_sb")
                for mc in range(2):
                    ps = pbig.tile([P, T], F32, tag="pb")
                    for kc in range(4):
                        nc.tensor.matmul(out=ps[:], lhsT=W["We2"][:, kc, mc, :],
                                         rhs=h_nn[:, kc, :], start=(kc == 0),
                                         stop=(kc == 3))
                    nc.vector.tensor_scalar_add(gcn_sb[:, mc, :], ps[:],
                                                W["be2"][:, mc:mc + 1])
                nc.sync.dma_start(out=dgcn[:, :, es], in_=gcn_sb[:])

                if stage < 1:
                    continue
                # ---- projections (head-major): Q(x_i), Ep(ef), Gp(geo), V(x_j)
                def proj(name, wname, bname, rhs_tile, nk, odt):
                    out_sb = wk.tile([P, 2, T], odt, tag=name)
                    for tc_ in range(2):
                        ps = pmid.tile([P, T], F32, tag="pm")
                        if nk == 1:
                            nc.tensor.matmul(out=ps[:], lhsT=W[wname][:, tc_, :],
                                             rhs=rhs_tile[:], start=True, stop=True)
                        else:
                            for kc in range(nk):
                                nc.tensor.matmul(out=ps[:],
                                                 lhsT=W[wname][:, kc, tc_, :],
                                                 rhs=rhs_tile[:, kc, :],
                                                 start=(kc == 0), stop=(kc == nk - 1))
                        nc.vector.tensor_scalar_add(out_sb[:, tc_, :], ps[:],
                                                    W[bname][:, tc_:tc_ + 1])
                    return out_sb

                Q = proj("Q", "Wq", "bq", xiT, 2, BF16)
                Ep = proj("Ep", "Wep", "bep", efT, 2, BF16)
                Gp = proj("Gp", "Wg", "bg", geoT, 1, BF16)
                V = proj("V", "Wv", "bv", xjT, 2, BF16)

                if stage < 2:
                    continue
                # ---- geo conv branch
                grelu = wk1.tile([96, 8, T], BF16, tag="crelu", name="grelu")
                for q in range(2):
                    pz = [pbig.tile([P, T], F32, tag="pb", name=f"pz{q}_{i}") for i in range(4)]
                    for b in range(4):
                        sl = slice(32 * b, 32 * b + 32)
                        tp = (32 * b, 0)
                        nc.tensor.matmul(out=pz[b][0:96], lhsT=W["Wg1q"][sl, :],
                                         rhs=Q[sl, q, :], start=True, stop=False,
                                         tile_position=tp)
                        nc.tensor.matmul(out=pz[b][0:96], lhsT=W["Wg1e"][sl, :],
                                         rhs=Ep[sl, q, :], start=False, stop=False,
                                         tile_position=tp)
                        nc.tensor.matmul(out=pz[b][0:96], lhsT=W["Wg1g"][sl, :],
                                         rhs=Gp[sl, q, :], start=False, stop=True,
                                         tile_position=tp)
                    for b in range(4):
                        h = q * 4 + b
                        if b % 2 == 0:
                            nc.scalar.activation(out=grelu[:, h, :], in_=pz[b][0:96],
                                                 func=AF.Relu, bias=W["bg1"][:])
                        else:
                            nc.vector.tensor_scalar(grelu[:, h, :], pz[b][0:96],
                                                    W["bg1"][:], 0.0,
                                                    op0=ALU.add, op1=ALU.max)
                expg = wk.tile([P, 2, T], BF16, tag="expg")
                for q in range(2):
                    pzz = pmid.tile([P, T], F32, tag="pm")
                    for b in range(4):
                        h = q * 4 + b
                        nc.tensor.matmul(out=pzz[32 * b:32 * b + 32], lhsT=W["Wg2T"][:],
                                         rhs=grelu[:, h, :], start=True, stop=True,
                                         tile_position=(0, 32 * b))
                    nc.scalar.activation(out=expg[:, q, :], in_=pzz[:], func=AF.Exp,
                                         bias=W["bg2r"][:])

                # ---- sem conv branch
                srelu = wk1.tile([64, 8, T], BF16, tag="crelu", name="srelu")
                for q in range(2):
                    pz = [pbig.tile([P, T], F32, tag="pb", name=f"pz{q}_{i}") for i in range(4)]
                    for b in range(4):
                        sl = slice(32 * b, 32 * b + 32)
                        tp = (32 * b, 0)
                        nc.tensor.matmul(out=pz[b][0:64], lhsT=W["Ws1q"][sl, :],
                                         rhs=Q[sl, q, :], start=True, stop=False,
                                         tile_position=tp)
                        nc.tensor.matmul(out=pz[b][0:64], lhsT=W["Ws1e"][sl, :],
                                         rhs=Ep[sl, q, :], start=False, stop=True,
                                         tile_position=tp)
                    for b in range(4):
                        h = q * 4 + b
                        if b % 2 == 0:
                            nc.scalar.activation(out=srelu[:, h, :], in_=pz[b][0:64],
                                                 func=AF.Relu, bias=W["bs1"][:])
                        else:
                            nc.vector.tensor_scalar(srelu[:, h, :], pz[b][0:64],
                                                    W["bs1"][:], 0.0,
                                                    op0=ALU.add, op1=ALU.max)
                exps = wk.tile([P, 2, T], BF16, tag="exps")
                for q in range(2):
                    pzz = pmid.tile([P, T], F32, tag="pm")
                    for b in range(4):
                        h = q * 4 + b
                        nc.tensor.matmul(out=pzz[32 * b:32 * b + 32], lhsT=W["Ws2T"][:],
                                         rhs=srelu[:, h, :], start=True, stop=True,
                                         tile_position=(0, 32 * b))
                    nc.scalar.activation(out=exps[:, q, :], in_=pzz[:], func=AF.Exp,
                                         bias=W["bs2r"][:])

                if stage < 3:
                    continue
                # ---- exp * v
                evg = wk.tile([P, 2, T], BF16, tag="evg")
                evs = wk.tile([P, 2, T], BF16, tag="evs")
                for q in range(2):
                    nc.vector.tensor_mul(out=evg[:, q, :], in0=expg[:, q, :],
                                         in1=V[:, q, :])
                    nc.vector.tensor_mul(out=evs[:, q, :], in0=exps[:, q, :],
                                         in1=V[:, q, :])

                # ---- per-(h,e) numerators/denominators via Sel matmuls
                pDg = pmid.tile([P, T], F32, tag="pm")
                pNg = pmid.tile([P, T], F32, tag="pm")
                pDs = pmid.tile([P, T], F32, tag="pm")
                pNs = pmid.tile([P, T], F32, tag="pm")
                for (pp, src) in ((pDg, expg), (pNg, evg), (pDs, exps), (pNs, evs)):
                    nc.tensor.matmul(out=pp[0:8], lhsT=W["SelA"][:], rhs=src[:, 0, :],
                                     start=True, stop=False)
                    nc.tensor.matmul(out=pp[0:8], lhsT=W["SelB"][:], rhs=src[:, 1, :],
                                     start=False, stop=True)
                cgcs = wk1.tile([8, 2, T], F32, tag="cgcs")
                rr = wk1.tile([8, 2, T], F32, tag="rr")
                nc.vector.reciprocal(out=rr[:, 0, :], in_=pDg[0:8])
                nc.vector.reciprocal(out=rr[:, 1, :], in_=pDs[0:8])
                nc.vector.tensor_mul(out=cgcs[:, 0, :], in0=pNg[0:8], in1=rr[:, 0, :])
                nc.vector.tensor_mul(out=cgcs[:, 1, :], in0=pNs[0:8], in1=rr[:, 1, :])
                pgo = pmid.tile([P, T], F32, tag="pm")
                nc.tensor.matmul(out=pgo[0:1], lhsT=W["ones8"][:], rhs=cgcs[:, 0, :],
                                 start=True, stop=True)
                pso = pmid.tile([P, T], F32, tag="pm")
                nc.tensor.matmul(out=pso[0:1], lhsT=W["ones8"][:], rhs=cgcs[:, 1, :],
                                 start=True, stop=True)

                if stage < 4:
                    continue
                # ---- balance MLP (from efT)
                pby = pmid.tile([P, T], F32, tag="pm")
                for kc in range(2):
                    nc.tensor.matmul(out=pby[:], lhsT=W["Wb1"][:, kc, 0, :],
                                     rhs=efT[:, kc, :], start=(kc == 0), stop=(kc == 1))
                y_sb = wk1.tile([P, T], F32, tag="y_sb")
                nc.vector.tensor_scalar_add(y_sb[:], pby[:], W["bb1"][:])
                ysq = wk1.tile([P, T], F32, tag="bscrA", name="ysq")
                nc.scalar.activation(out=ysq[:], in_=y_sb[:], func=AF.Square)
                pm1 = pmid.tile([P, T], F32, tag="pm")
                nc.tensor.matmul(out=pm1[0:1], lhsT=W["ones128d"][:], rhs=y_sb[:],
                                 start=True, stop=True)
                pm2 = pmid.tile([P, T], F32, tag="pm")
                nc.tensor.matmul(out=pm2[0:1], lhsT=W["ones128d"][:], rhs=ysq[:],
                                 start=True, stop=True)
                sm1 = wk1.tile([1, T], F32, tag="sm1")
                nc.vector.tensor_copy(out=sm1[:], in_=pm1[0:1])
                var = wk1.tile([1, T], F32, tag="var")
                nc.vector.tensor_tensor(out=var[:], in0=sm1[:], in1=sm1[:], op=ALU.mult)
                nc.vector.tensor_tensor(out=var[:], in0=pm2[0:1], in1=var[:],
                                        op=ALU.subtract)
                nc.vector.tensor_scalar_add(var[:], var[:], 1e-5)
                stdv = wk1.tile([1, T], F32, tag="stdv")
                nc.scalar.activation(out=stdv[:], in_=var[:], func=AF.Sqrt, bias=0.0)
                rstd = wk1.tile([1, T], F32, tag="rstd")
                nc.vector.reciprocal(out=rstd[:], in_=stdv[:])
                pmb = pmid.tile([P, T], F32, tag="pm")
                nc.tensor.matmul(out=pmb[:], lhsT=W["onesbc"][:], rhs=sm1[:],
                                 start=True, stop=True)
                prb = pmid.tile([P, T], F32, tag="pm")
                nc.tensor.matmul(out=prb[:], lhsT=W["onesbc"][:], rhs=rstd[:],
                                 start=True, stop=True)
                nc.vector.tensor_tensor(out=y_sb[:], in0=y_sb[:], in1=pmb[:],
                                        op=ALU.subtract)
                nc.vector.tensor_tensor(out=y_sb[:], in0=y_sb[:], in1=prb[:], op=ALU.mult)
                h1 = wk1.tile([P, T], F32, tag="bscrB", name="h1")
                nc.scalar.activation(out=h1[:], in_=y_sb[:], func=AF.Relu,
                                     bias=W["beta1"][:], scale=W["g1"][:])
                # LN2 (64)
                ph2 = pmid.tile([P, T], F32, tag="pm")
                nc.tensor.matmul(out=ph2[0:64], lhsT=W["Wb2"][:], rhs=h1[:],
                                 start=True, stop=True)
                y2 = wk1.tile([64, T], F32, tag="y2")
                nc.vector.tensor_scalar_add(y2[:], ph2[0:64], W["bb2"][:])
                y2q = wk1.tile([64, T], F32, tag="bscrA", name="y2q")
                nc.scalar.activation(out=y2q[:], in_=y2[:], func=AF.Square)
                pm3 = pmid.tile([P, T], F32, tag="pm")
                nc.tensor.matmul(out=pm3[0:1], lhsT=W["ones64d"][:], rhs=y2[:],
                                 start=True, stop=True)
                pm4 = pmid.tile([P, T], F32, tag="pm")
                nc.tensor.matmul(out=pm4[0:1], lhsT=W["ones64d"][:], rhs=y2q[:],
                                 start=True, stop=True)
                sm3 = wk1.tile([1, T], F32, tag="sm1", name="sm3")
                nc.vector.tensor_copy(out=sm3[:], in_=pm3[0:1])
                var2 = wk1.tile([1, T], F32, tag="var", name="var2")
                nc.vector.tensor_tensor(out=var2[:], in0=sm3[:], in1=sm3[:], op=ALU.mult)
                nc.vector.tensor_tensor(out=var2[:], in0=pm4[0:1], in1=var2[:],
                                        op=ALU.subtract)
                nc.vector.tensor_scalar_add(var2[:], var2[:], 1e-5)
                stdv2 = wk1.tile([1, T], F32, tag="stdv", name="stdv2")
                nc.scalar.activation(out=stdv2[:], in_=var2[:], func=AF.Sqrt, bias=0.0)
                rstd2 = wk1.tile([1, T], F32, tag="rstd", name="rstd2")
                nc.vector.reciprocal(out=rstd2[:], in_=stdv2[:])
                pmb2 = pmid.tile([P, T], F32, tag="pm")
                nc.tensor.matmul(out=pmb2[0:64], lhsT=W["onesbc"][:1, 0:64], rhs=sm3[:],
                                 start=True, stop=True)
                prb2 = pmid.tile([P, T], F32, tag="pm")
                nc.tensor.matmul(out=prb2[0:64], lhsT=W["onesbc"][:1, 0:64], rhs=rstd2[:],
                                 start=True, stop=True)
                nc.vector.tensor_tensor(out=y2[:], in0=y2[:], in1=pmb2[0:64],
                                        op=ALU.subtract)
                nc.vector.tensor_tensor(out=y2[:], in0=y2[:], in1=prb2[0:64],
                                        op=ALU.mult)
                h2 = wk1.tile([64, T], F32, tag="bscrB", name="h2")
                nc.scalar.activation(out=h2[:], in_=y2[:], func=AF.Relu,
                                     bias=W["beta2"][:], scale=W["g2"][:])
                pb3 = pmid.tile([P, T], F32, tag="pm")
                nc.tensor.matmul(out=pb3[0:1], lhsT=W["Wb3"][:], rhs=h2[:],
                                 start=True, stop=True)
                b_sb = wk1.tile([1, T], F32, tag="b_sb")
                nc.scalar.activation(out=b_sb[:], in_=pb3[0:1], func=AF.Sigmoid,
                                     bias=W["bb3t"][:])
                nc.sync.dma_start(out=dbal[g:g + 1, :], in_=b_sb[0:1, :])

                # ---- s = sem + b*(geo - sem)
                sem_sb = wk1.tile([1, T], F32, tag="sem_sb")
                nc.vector.tensor_copy(out=sem_sb[:], in_=pso[0:1])
                dsb = wk1.tile([1, T], F32, tag="dsb")
                nc.vector.tensor_tensor(out=dsb[:], in0=pgo[0:1], in1=sem_sb[:],
                                        op=ALU.subtract)
                nc.vector.tensor_tensor(out=dsb[:], in0=b_sb[:], in1=dsb[:],
                                        op=ALU.mult)
                s_sb = wk1.tile([1, T], F32, tag="s_sb")
                nc.vector.tensor_tensor(out=s_sb[:], in0=dsb[:], in1=sem_sb[:],
                                        op=ALU.add)
                nc.sync.dma_start(out=dsdbg[g:g + 1, :], in_=s_sb[0:1, :])
                pst = pmid.tile([P, T], F32, tag="pm", name=f"pst{g}")
                for k in range(4):
                    nc.tensor.matmul(out=pst[:, k:k + 1],
                                     lhsT=s_sb[0:1, k * P:(k + 1) * P],
                                     rhs=W["ones1"][:], start=True, stop=True)
                nc.vector.tensor_copy(out=s_pm[:, g * 4:(g + 1) * 4], in_=pst[:, 0:4])

            # ---- segmax
            if stage >= 5:
                offs_t = acc.tile([P, J, C], F32, name="offs_t")
                nc.sync.dma_start(out=offs_t[:], in_=doffs[:])
                g_pm = acc.tile([P, J], F32, name="g_pm")
                scr = acc.tile([P, C], F32, name="scr")
                for j in range(J):
                    nc.vector.tensor_tensor(out=scr[:], in0=s_pm[:],
                                            in1=offs_t[:, j, :], op=ALU.add)
                    nc.vector.reduce_max(out=g_pm[:, j:j + 1], in_=scr[:],
                                         axis=mybir.AxisListType.X)
                gmask = acc.tile([P, J], F32, name="gmask")
                nc.vector.tensor_scalar(gmask[:], g_pm[:], -1.0e29, None, op0=ALU.is_ge)
                nc.vector.tensor_tensor(out=g_pm[:], in0=g_pm[:], in1=gmask[:], op=ALU.mult)
                g_row = acc.tile([1, NP], F32, name="g_row")
                for j in range(J):
                    prj = pmid.tile([P, T], F32, tag="pm", name=f"prj{j}")
                    nc.tensor.matmul(out=prj[0:1, 0:P], lhsT=g_pm[:, j:j + 1],
                                     rhs=W["ident"][:], start=True, stop=True)
                    nc.vector.tensor_copy(out=g_row[0:1, j * P:(j + 1) * P],
                                          in_=prj[0:1, 0:P])

            if stage >= 5:
                nc.sync.dma_start(out=dgrow[:], in_=g_row[:])
            # ---- prop MLP over node groups
            if stage >= 6:
                for ng in range(GP):
                    ns = slice(ng * T, (ng + 1) * T)
                    xpt = io.tile([P, 2, T], F32, tag="xpt")
                    nc.sync.dma_start(out=xpt[:], in_=dxp[:, :, ns])
                    hp = wk.tile([P, 4, T], F32, tag="hp")
                    for mc in range(4):
                        ps = pbig.tile([P, T], F32, tag="pb")
                        nc.tensor.matmul(out=ps[:], lhsT=W["Wp1a"][:, 0, mc, :],
                                         rhs=xpt[:, 0, :], start=True, stop=False)
                        nc.tensor.matmul(out=ps[:], lhsT=W["Wp1a"][:, 1, mc, :],
                                         rhs=xpt[:, 1, :], start=False, stop=False)
                        nc.tensor.matmul(out=ps[:], lhsT=W["wp1sum"][:, mc, :],
                                         rhs=g_row[:, ns], start=False, stop=True)
                        nc.scalar.activation(out=hp[:, mc, :], in_=ps[:], func=AF.Relu,
                                             bias=W["bp1"][:, mc:mc + 1])
                    xxsb = wk.tile([P, 2, T], F32, tag="xxsb")
                    for mc in range(2):
                        ps = pbig.tile([P, T], F32, tag="pb")
                        for kc in range(4):
                            nc.tensor.matmul(out=ps[:], lhsT=W["Wp2"][:, kc, mc, :],
                                             rhs=hp[:, kc, :], start=(kc == 0),
                                             stop=(kc == 3))
                        nc.vector.tensor_scalar_add(xxsb[:, mc, :], ps[:],
                                                    W["bp2"][:, mc:mc + 1])
                    nc.sync.dma_start(out=dxx[:, :, ns], in_=xxsb[:])

    nc.compile()
    return nc


# --------------------------------------------------------------------------
# entry point
# --------------------------------------------------------------------------

def kernel(**inputs):
    inp = {k: np.asarray(v) for k, v in inputs.items()}
    idx_i = inp["edge_index"][0].astype(np.int64)
    idx_j = inp["edge_index"][1].astype(np.int64)
    x32 = inp["x"].astype(np.float32)

    order, starts, deg, cores, C, J = _plan(idx_i)
    EC = P * C

    import os as _os
    stage = int(_os.environ.get("KSTAGE", "99"))
    key = (EC, J, C, stage)
    if key not in _cache:
        _cache[key] = _build(EC, J, C, stage)
    nc = _cache[key]

    w = _prep_weights(inp)
    x16 = x32.astype(ml_dtypes.bfloat16)

    in_maps = []
    metas = []
    for c in range(8):
        d = _prep_core(cores[c], order, starts, deg, C, J, idx_i, idx_j,
                       inp["edge_feature"].astype(np.float32),
                       inp["geo_features"].astype(np.float32), x32)
        metas.append(d)
        m = {"x16": x16, "xi16": d["xi16"], "xj16": d["xj16"],
             "efT": d["efT"], "geoT": d["geoT"], "offs": d["offs"],
             "xpropT": d["xpropT"]}
        for nm in w:
            m[nm] = np.ascontiguousarray(w[nm])
        in_maps.append(m)

    import os
    trace = bool(os.environ.get("BASS_TRACE"))
    res = bass_utils.run_bass_kernel_spmd(nc, in_maps, core_ids=list(range(8)),
                                          trace=trace)
    global _last_results, _last_metas
    _last_results = res
    _last_metas = metas

    gcn = np.zeros((E, DE), np.float32)
    balance = np.zeros(E, np.float32)
    xx = np.zeros((N, DN), np.float32)
    for c in range(8):
        r = res.results[c]
        d = metas[c]
        ecn = d["dev_edges"].shape[0]
        gcnT = r["gcnT"].transpose(1, 0, 2).reshape(256, ecn)
        gcn[d["dev_edges"]] = gcnT.T
        balance[d["dev_edges"]] = r["bal"].reshape(-1)
        xxT = r["xxT"].transpose(1, 0, 2).reshape(256, -1).T  # (NP, 256)
        sel = d["prop_nodes"] >= 0
        xx[d["prop_nodes"][sel]] = xxT[sel]
    return xx, gcn, balance
